# revision 15
# baseline (speedup 1.0000x reference)
"""Multi-head self-attention (B=4,S=2048,D=1024,H=16,DH=64, causal) on 8 trn2 cores.

Sharding: core c -> batch b=c//2, head-group g=c%2 (8 heads each).

v2 restructure over the 312us baseline (PE busy was 268us with cold-clock
inflation, 19.4us DMA head, serial qb-boundary stalls):
- DMA head: x block + weights split across 5 engine DMA queues in parallel
  (was serial on one queue; first matmul at 19.4us -> target ~9us).
- Causal-mask matmuls pair-combined: one [128x256] MM per diagonal PSUM
  pair via a strided 3D out AP (was 2x [128x128] MMs).
- Emission engine: QKV-projection and output-projection work is split into
  ~0.9us pieces held in a filler deque and injected between attention
  u-rounds, keeping PE dense while ACT (exp) catches up; PRE-drains before
  each head-pair guarantee dependencies.
- Scores in bf16 (fp8 fails numerics: e4m3 anywhere pushes rel err to
  3e-2..8e-2 vs the 2e-2 gate; measured in numpy sim).
- exp on ACT batched over k-tile pairs ([128,1024] two-bank PSUM reads).
- attn@V in bf16 with ones-column denominator; renorm via
  reciprocal_approx_fast (DVE) + partition_broadcast (gpsimd) + mult (DVE).
- bv and bp folded host-side into the output-projection bias.

K-projection quirk (reference views k as (B,S,DH,H)): head h uses Wk rows
[dh*16+h for dh in range(64)] -- handled by host-side row gather.
"""
import numpy as np
import ml_dtypes
from collections import deque

import concourse.mybir as mybir
import concourse.tile as tile
from concourse import bacc
from concourse.ap import AP
from concourse.bass_utils import run_bass_kernel_spmd

F32 = mybir.dt.float32
BF16 = mybir.dt.bfloat16
AF = mybir.ActivationFunctionType
ALU = mybir.AluOpType

B, S, D, H, DH = 4, 2048, 1024, 16, 64
FG = 512          # features per head-group (8 heads * 64)
N_CORES = 8
SCALE = 0.125     # 1/sqrt(64)

_NC = None


def _build():
    nc = bacc.Bacc("TRN2", target_bir_lowering=False, debug=False,
                   num_devices=N_CORES, enable_asserts=False)
    xbT_d = nc.dram_tensor("xbT", [D, S], BF16, kind="ExternalInput").ap()
    wqT_d = nc.dram_tensor("wqT", [D, FG], BF16, kind="ExternalInput").ap()
    wkT_d = nc.dram_tensor("wkT", [D, FG], BF16, kind="ExternalInput").ap()
    wvT_d = nc.dram_tensor("wvT", [D, FG], BF16, kind="ExternalInput").ap()
    wpT_d = nc.dram_tensor("wpT", [FG, D], BF16, kind="ExternalInput").ap()
    bqs_d = nc.dram_tensor("bqs", [128, 4], F32, kind="ExternalInput").ap()
    bks_d = nc.dram_tensor("bks", [128, 4], F32, kind="ExternalInput").ap()
    bps_d = nc.dram_tensor("bps", [128, 8], F32, kind="ExternalInput").ap()
    negI_d = nc.dram_tensor("negI", [128, 128], BF16, kind="ExternalInput").ap()
    ltB2_d = nc.dram_tensor("ltB2", [128, 256], BF16, kind="ExternalInput").ap()
    out_d = nc.dram_tensor("outT", [D, S], F32, kind="ExternalOutput").ap()

    with tile.TileContext(nc) as tc:
        with tc.tile_pool(name="persist", bufs=1) as pp, \
             tc.tile_pool(name="xin", bufs=2) as xp, \
             tc.tile_pool(name="etile", bufs=6) as ep, \
             tc.tile_pool(name="small", bufs=4) as sp, \
             tc.tile_pool(name="outtile", bufs=4) as op, \
             tc.tile_pool(name="pspair", bufs=3, space="PSUM") as ps_pair, \
             tc.tile_pool(name="psot", bufs=2, space="PSUM") as ps_ot:

            # ---- persistent SBUF tensors ----
            wq = pp.tile([128, 8, FG], BF16)   # [dp, do, f]  (pre-scaled 1/8)
            wk = pp.tile([128, 8, FG], BF16)
            wv = pp.tile([128, 8, FG], BF16)
            wp = pp.tile([128, 4, D], BF16)    # [cp, co, j]
            qt = pp.tile([128, 4, S], BF16)    # [fp, fo, s]
            kt = pp.tile([128, 4, S], BF16)
            va = pp.tile([128, 16, 8, DH + 1], BF16)  # [skp, sko, h, dh|1]
            on_ = pp.tile([128, 4, S], BF16)   # renormed out^T  [cp, co, s]
            negI = pp.tile([128, 128], BF16)
            ltB2 = pp.tile([128, 256], BF16)
            bqs = pp.tile([128, 4], F32)
            bks = pp.tile([128, 4], F32)
            bps = pp.tile([128, 8], F32)

            xbT_r = xbT_d.rearrange("(do dp) s -> dp do s", dp=128)
            wq_r = wqT_d.rearrange("(do dp) f -> dp do f", dp=128)
            wk_r = wkT_d.rearrange("(do dp) f -> dp do f", dp=128)
            wv_r = wvT_d.rearrange("(do dp) f -> dp do f", dp=128)
            wp_r = wpT_d.rearrange("(co cp) j -> cp co j", cp=128)
            xtiles = {}

            def emit_xdma(sb, split=False):
                xb = xp.tile([128, 8, 512], BF16)
                ssl = slice(sb * 512, (sb + 1) * 512)
                if split:  # xb0 latency-critical: both HW rings
                    nc.sync.dma_start(xb[:, 0:4, :], xbT_r[:, 0:4, ssl])
                    nc.scalar.dma_start(xb[:, 4:8, :], xbT_r[:, 4:8, ssl])
                else:
                    nc.gpsimd.dma_start(xb[:], xbT_r[:, :, ssl])
                xtiles[sb] = xb

            # only 3 DMA queues exist: gpsimd (software DGE; slow to start,
            # ~1us per descriptor batch), sync(SP) and scalar(Activation)
            # (hardware DGE, live at ~2.5us).  The 16 DMA engines drain all
            # rings round-robin at descriptor granularity, so aggregate
            # bandwidth (~440GB/s) is shared by whatever is in flight: the
            # lever for latency is ring ORDER.  Critical prefix xb0+wq
            # first on the two HW rings, then wk, wv, wp; the gpsimd ring
            # only prefetches xb1 (xb2/xb3 deferred by xin bufs=2
            # back-pressure).
            nc.scalar.dma_start(bqs[:], bqs_d[:])
            nc.scalar.dma_start(bks[:], bks_d[:])
            nc.scalar.dma_start(negI[:], negI_d[:])
            nc.scalar.dma_start(ltB2[:], ltB2_d[:])
            nc.scalar.dma_start(bps[:], bps_d[:])
            emit_xdma(0, split=True)
            nc.sync.dma_start(wq[:, 0:4, :], wq_r[:, 0:4, :])
            nc.scalar.dma_start(wq[:, 4:8, :], wq_r[:, 4:8, :])
            nc.sync.dma_start(wk[:, 0:4, :], wk_r[:, 0:4, :])
            nc.scalar.dma_start(wk[:, 4:8, :], wk_r[:, 4:8, :])
            nc.sync.dma_start(wv[:, 0:4, :], wv_r[:, 0:4, :])
            nc.scalar.dma_start(wv[:, 4:8, :], wv_r[:, 4:8, :])
            nc.sync.dma_start(wp[:, 0:2, :], wp_r[:, 0:2, :])
            nc.scalar.dma_start(wp[:, 2:4, :], wp_r[:, 2:4, :])
            nc.vector.memset(va[:, :, :, DH:DH + 1], 1.0)

            # ---- filler: split B/proj work into ~0.9us pieces ----
            done = set()
            fq = deque()
            bslot = {}

            def qk_piece(sb, which, ft, half):
                w_sb, dst, bias = ((wq, qt, bqs) if which == 'q'
                                   else (wk, kt, bks))
                key = (which, sb, ft)
                if half == 0:
                    bslot[key] = ps_pair.tile([128, 1024], F32, space="PSUM",
                                              tag="pair", name="pst")
                ps = bslot[key][:, 0:512]
                for do in (range(0, 4) if half == 0 else range(4, 8)):
                    nc.tensor.matmul(
                        ps, w_sb[:, do, ft * 128:(ft + 1) * 128],
                        xtiles[sb][:, do, :], start=(do == 0), stop=(do == 7))
                if half == 1:
                    nc.vector.tensor_scalar_add(
                        dst[:, ft, sb * 512:(sb + 1) * 512], ps,
                        bias[:, ft:ft + 1])
                    del bslot[key]

            def v_piece(sb, st, half):
                key = ('v', sb, st)
                if half == 0:
                    bslot[key] = ps_pair.tile([128, 1024], F32, space="PSUM",
                                              tag="pair", name="pst")
                ps = bslot[key][:, 0:512]
                for do in (range(0, 4) if half == 0 else range(4, 8)):
                    nc.tensor.matmul(
                        ps, xtiles[sb][:, do, st * 128:(st + 1) * 128],
                        wv[:, do, :], start=(do == 0), stop=(do == 7))
                if half == 1:
                    nc.vector.tensor_copy(
                        va[:, sb * 4 + st, :, :DH],
                        ps.rearrange("p (h d) -> p h d", h=8))
                    del bslot[key]

            def proj_piece(qb, jt):
                pst = ps_pair.tile([128, 1024], F32, space="PSUM", tag="pair")
                psj = pst[:, 0:512]
                for co in range(4):
                    nc.tensor.matmul(
                        psj, wp[:, co, jt * 128:(jt + 1) * 128],
                        on_[:, co, qb * 512:(qb + 1) * 512],
                        start=(co == 0), stop=(co == 3))
                ot_sb = op.tile([128, 512], F32, tag="o")
                nc.vector.tensor_scalar_add(ot_sb[:], psj, bps[:, jt:jt + 1])
                nc.sync.dma_start(
                    out_d[jt * 128:(jt + 1) * 128, qb * 512:(qb + 1) * 512],
                    ot_sb[:])

            def add_qk(sb, ft):
                for which in ('q', 'k'):
                    fq.append((None, lambda s=sb, w=which, f=ft:
                               qk_piece(s, w, f, 0)))
                    fq.append(((which, sb, ft), lambda s=sb, w=which, f=ft:
                               qk_piece(s, w, f, 1)))

            def add_v(sb):
                for st in range(4):
                    fq.append((None, lambda s=sb, t=st: v_piece(s, t, 0)))
                    fq.append((('v', sb, st) if st < 3 else ('v', sb),
                               lambda s=sb, t=st: v_piece(s, t, 1)))

            def pop_emit(n):
                for _ in range(n):
                    if not fq:
                        return
                    key, fn = fq.popleft()
                    fn()
                    if key is not None:
                        done.add(key)

            def drain_until(keys):
                while not all(k in done for k in keys):
                    assert fq, f"filler empty but need {keys}"
                    pop_emit(1)

            # ---- bf16 score matmul for head h, k-tile t, q-block qb ----
            def mm_score(out_ap, h, t, qb, c0, start, stop):
                g2, j = h % 2, h // 2
                p0 = 64 * g2
                lhsT = kt[p0:p0 + 64, j, 128 * t:128 * t + 128]
                rhs = qt[p0:p0 + 64, j, qb * 512 + c0:(qb + 1) * 512]
                nc.tensor.matmul(out_ap, lhsT, rhs, start=start, stop=stop)

            # combined causal-mask MM: adds -30*ltB to the two diagonal
            # 128-col strips of the score pair tile in one N=256 matmul.
            # Strip offsets within the [128,1024] pair tile are 128*m0 and
            # 512+128*(m0+1) -> constant spacing 640.
            def emit_mask_pair(pt, m0):
                base = pt[:, 128 * m0:128 * m0 + 128]
                out_ap = AP(base.tensor, base.offset,
                            [[1024, 128], [640, 2], [1, 128]])
                nc.tensor.matmul(out_ap, negI[:], ltB2[:],
                                 start=False, stop=True,
                                 skip_group_check=True)

            # ---- attention for (q-block qb, head h) ----
            def emit_av(ot, qb, h, et, u, nt):
                for half in range(2):
                    t = 2 * u + half
                    m = t - 4 * qb
                    c0 = 0 if m < 0 else 128 * m
                    hsl = 512 * half
                    nc.tensor.matmul(
                        ot[0:DH + 1, c0:512], va[:, t, h, :],
                        et[:, hsl + c0:hsl + 512],
                        start=(t == 0), stop=(t == nt - 1),
                        skip_group_check=True)

            def gen_c(qb, h):
                nt = 4 * qb + 4
                qsl = slice(qb * 512, (qb + 1) * 512)
                ot = None  # allocated lazily at first attn@V (avoids a
                # boundary stall on the ot-pool WAR with the previous
                # head-pair's pending renorm)
                pend = deque()  # (et, u) with attn@V deferred two rounds
                for u in range(nt // 2):
                    pt = ps_pair.tile([128, 1024], F32, space="PSUM", tag="pair")
                    et = ep.tile([128, 1024], BF16, tag="e")
                    m0 = 2 * u - 4 * qb
                    for half in range(2):
                        t = 2 * u + half
                        m = t - 4 * qb
                        hsl = 512 * half
                        if m < 0:  # full tile
                            mm_score(pt[:, hsl:hsl + 512], h, t, qb, 0,
                                     True, True)
                        else:      # diagonal tile: trim cols, mask strip later
                            c0 = 128 * m
                            mm_score(pt[:, hsl + c0:hsl + 512], h, t, qb, c0,
                                     True, False)
                    if m0 >= 0:
                        emit_mask_pair(pt, m0)
                    yield
                    # exp (ACT), batched over the pair when both halves full
                    if m0 < 0:
                        nc.scalar.activation(et[:], pt[:], AF.Exp)
                    else:
                        c0a, c0b = 128 * m0, 128 * (m0 + 1)
                        nc.scalar.activation(
                            et[:, c0a:512], pt[:, c0a:512], AF.Exp)
                        nc.scalar.activation(
                            et[:, 512 + c0b:1024], pt[:, 512 + c0b:1024],
                            AF.Exp)
                    pend.append((et, u))
                    if len(pend) > 2:
                        if ot is None:
                            ot = ps_ot.tile([DH + 1, 512], F32, space="PSUM",
                                            tag="ot")
                        e0, u0 = pend.popleft()
                        emit_av(ot, qb, h, e0, u0, nt)
                    yield
                while pend:
                    if ot is None:
                        ot = ps_ot.tile([DH + 1, 512], F32, space="PSUM",
                                        tag="ot")
                    e0, u0 = pend.popleft()
                    emit_av(ot, qb, h, e0, u0, nt)
                # softmax renorm: divide by ones-column row of ot
                dn = sp.tile([1, 512], F32, tag="dn")
                nc.vector.tensor_copy(dn[:], ot[DH:DH + 1, :])
                rec = sp.tile([1, 512], F32, tag="rec")
                nc.vector.reciprocal_approx_fast(rec[:], dn[:])
                rb = sp.tile([DH, 512], F32, tag="rb")
                nc.gpsimd.partition_broadcast(rb[:], rec[:])
                r0 = 64 * (h % 2)
                dst = on_[r0:r0 + 64, h // 2, qsl]
                nc.vector.tensor_tensor(dst, ot[0:DH, :], rb[:], ALU.mult)

            slot = {'i': 0, 'skip': 0, 'every': 1, 'pop': 1}

            def emit_c_pair(qb, h0, h1):
                gens = [gen_c(qb, h0), gen_c(qb, h1)]
                alive = [True, True]
                step = 0
                while any(alive):
                    for i in (0, 1):
                        if alive[i]:
                            try:
                                next(gens[i])
                            except StopIteration:
                                alive[i] = False
                    step += 1
                    if step % 2 == 0:
                        slot['i'] += 1
                        if (slot['i'] > slot['skip']
                                and (slot['i'] - slot['skip']) % slot['every'] == 0):
                            pop_emit(slot['pop'])

            # ---- emission schedule ----
            # critical prefix of B(0): all four Q chains first (bridges PE
            # over the wk/wv DMA arrival), then K ft0 and V
            for ft in range(4):
                qk_piece(0, 'q', ft, 0); qk_piece(0, 'q', ft, 1)
                done.add(('q', 0, ft))
            qk_piece(0, 'k', 0, 0); qk_piece(0, 'k', 0, 1); done.add(('k', 0, 0))
            for st in range(4):
                v_piece(0, st, 0); v_piece(0, st, 1)
            done.add(('v', 0))
            for ft in range(1, 4):
                fq.append((None, lambda f=ft: qk_piece(0, 'k', f, 0)))
                fq.append((('k', 0, ft), lambda f=ft: qk_piece(0, 'k', f, 1)))

            # injection config per qb: skip slots after the qb boundary so
            # injected pieces never stall the in-order PE queue on fresh
            # renorm/DMA deps
            inject = {0: (0, 1, 3), 1: (2, 1, 2), 2: (2, 1, 2), 3: (2, 4, 1)}
            for qb in range(4):
                slot['i'] = 0
                slot['skip'], slot['every'], slot['pop'] = inject[qb]
                if qb >= 1:  # proj pieces first: safe once prev qb renormed
                    for jt in range(8):
                        fq.append((None, lambda q=qb - 1, j=jt:
                                   proj_piece(q, j)))
                if qb < 3:
                    emit_xdma(qb + 1)
                    for ft in range(4):
                        add_qk(qb + 1, ft)
                        if ft == 0:
                            add_v(qb + 1)
                for hp in range(4):
                    need = [('q', qb, hp), ('k', qb, hp)]
                    if hp == 0:
                        need.append(('v', qb))
                    drain_until(need)
                    emit_c_pair(qb, 2 * hp, 2 * hp + 1)
            pop_emit(len(fq))
            for jt in range(8):
                proj_piece(3, jt)

    nc.compile()
    return nc


def kernel(x, Wq, bq, Wk, bk, Wv, bv, Wp, bp):
    global _NC
    if _NC is None:
        _NC = _build()

    x = np.asarray(x, np.float32)
    Wq, bq = np.asarray(Wq, np.float32), np.asarray(bq, np.float32)
    Wk, bk = np.asarray(Wk, np.float32), np.asarray(bk, np.float32)
    Wv, bv = np.asarray(Wv, np.float32), np.asarray(bv, np.float32)
    Wp, bp = np.asarray(Wp, np.float32), np.asarray(bp, np.float32)

    bf = ml_dtypes.bfloat16
    negI = np.ascontiguousarray((-30.0 * np.eye(128, dtype=np.float32)).astype(bf))
    i_ = np.arange(128)
    ltB = (i_[None, :] < i_[:, None]).astype(np.float32).astype(bf)
    ltB2 = np.ascontiguousarray(np.concatenate([ltB, ltB], axis=1))

    xbT = [np.ascontiguousarray(x[b].T.astype(bf)) for b in range(B)]

    in_maps = []
    for c in range(N_CORES):
        b, g = c // 2, c % 2
        hs = range(8 * g, 8 * g + 8)
        kidx = np.array([dh * 16 + h for h in hs for dh in range(DH)])
        fsl = slice(FG * g, FG * (g + 1))
        bp_c = (bp if g == 0 else 0.0) + Wp[:, fsl] @ bv[fsl]
        in_maps.append({
            "xbT": xbT[b],
            "wqT": np.ascontiguousarray((SCALE * Wq[fsl].T).astype(bf)),
            "wkT": np.ascontiguousarray(Wk[kidx].T.astype(bf)),
            "wvT": np.ascontiguousarray(Wv[fsl].T.astype(bf)),
            "wpT": np.ascontiguousarray(Wp[:, fsl].T.astype(bf)),
            "bqs": np.ascontiguousarray((SCALE * bq[fsl]).reshape(4, 128).T),
            "bks": np.ascontiguousarray(bk[kidx].reshape(4, 128).T),
            "bps": np.ascontiguousarray(bp_c.reshape(8, 128).T.astype(np.float32)),
            "negI": negI,
            "ltB2": ltB2,
        })

    res = run_bass_kernel_spmd(_NC, in_maps, core_ids=list(range(N_CORES)))
    out = np.empty((B, S, D), np.float32)
    for b in range(B):
        acc = res.results[2 * b]["outT"] + res.results[2 * b + 1]["outT"]
        out[b] = acc.T
    return out


# revision 18
# speedup vs baseline: 1.0571x; 1.0571x over previous
"""Multi-head self-attention (B=4,S=2048,D=1024,H=16,DH=64, causal) on 8 trn2 cores.

Sharding: core c -> batch b=c//2, head-group g=c%2 (8 heads each).

v2 restructure over the 312us baseline (PE busy was 268us with cold-clock
inflation, 19.4us DMA head, serial qb-boundary stalls):
- DMA head: x block + weights split across 5 engine DMA queues in parallel
  (was serial on one queue; first matmul at 19.4us -> target ~9us).
- Causal-mask matmuls pair-combined: one [128x256] MM per diagonal PSUM
  pair via a strided 3D out AP (was 2x [128x128] MMs).
- Emission engine: QKV-projection and output-projection work is split into
  ~0.9us pieces held in a filler deque and injected between attention
  u-rounds, keeping PE dense while ACT (exp) catches up; PRE-drains before
  each head-pair guarantee dependencies.
- Scores in bf16 (fp8 fails numerics: e4m3 anywhere pushes rel err to
  3e-2..8e-2 vs the 2e-2 gate; measured in numpy sim).
- exp on ACT batched over k-tile pairs ([128,1024] two-bank PSUM reads).
- attn@V in bf16 with ones-column denominator; renorm via
  reciprocal_approx_fast (DVE) + partition_broadcast (gpsimd) + mult (DVE).
- bv and bp folded host-side into the output-projection bias.

K-projection quirk (reference views k as (B,S,DH,H)): head h uses Wk rows
[dh*16+h for dh in range(64)] -- handled by host-side row gather.
"""
import numpy as np
import ml_dtypes
from collections import deque

import concourse.mybir as mybir
import concourse.tile as tile
from concourse import bacc
from concourse.ap import AP
from concourse.bass_utils import run_bass_kernel_spmd

F32 = mybir.dt.float32
BF16 = mybir.dt.bfloat16
AF = mybir.ActivationFunctionType
ALU = mybir.AluOpType

B, S, D, H, DH = 4, 2048, 1024, 16, 64
FG = 512          # features per head-group (8 heads * 64)
N_CORES = 8
SCALE = 0.125     # 1/sqrt(64)

_NC = None


def _build():
    nc = bacc.Bacc("TRN2", target_bir_lowering=False, debug=False,
                   num_devices=N_CORES, enable_asserts=False)
    xbT_d = nc.dram_tensor("xbT", [D, S], BF16, kind="ExternalInput").ap()
    wqT_d = nc.dram_tensor("wqT", [D, FG], BF16, kind="ExternalInput").ap()
    wkT_d = nc.dram_tensor("wkT", [D, FG], BF16, kind="ExternalInput").ap()
    wvT_d = nc.dram_tensor("wvT", [D, FG], BF16, kind="ExternalInput").ap()
    wpT_d = nc.dram_tensor("wpT", [FG, D], BF16, kind="ExternalInput").ap()
    bqs_d = nc.dram_tensor("bqs", [128, 4], F32, kind="ExternalInput").ap()
    bks_d = nc.dram_tensor("bks", [128, 4], F32, kind="ExternalInput").ap()
    bps_d = nc.dram_tensor("bps", [128, 8], F32, kind="ExternalInput").ap()
    negI_d = nc.dram_tensor("negI", [128, 128], BF16, kind="ExternalInput").ap()
    ltB2_d = nc.dram_tensor("ltB2", [128, 256], BF16, kind="ExternalInput").ap()
    out_d = nc.dram_tensor("outT", [D, S], F32, kind="ExternalOutput").ap()

    with tile.TileContext(nc) as tc:
        with tc.tile_pool(name="persist", bufs=1) as pp, \
             tc.tile_pool(name="xin", bufs=2) as xp, \
             tc.tile_pool(name="etile", bufs=8) as ep, \
             tc.tile_pool(name="small", bufs=4) as sp, \
             tc.tile_pool(name="outtile", bufs=4) as op, \
             tc.tile_pool(name="pspair", bufs=3, space="PSUM") as ps_pair, \
             tc.tile_pool(name="psot", bufs=2, space="PSUM") as ps_ot:

            # ---- persistent SBUF tensors ----
            wq = pp.tile([128, 8, FG], BF16)   # [dp, do, f]  (pre-scaled 1/8)
            wk = pp.tile([128, 8, FG], BF16)
            wv = pp.tile([128, 8, FG], BF16)
            wp = pp.tile([128, 4, D], BF16)    # [cp, co, j]
            qt = pp.tile([128, 4, S], BF16)    # [fp, fo, s]
            kt = pp.tile([128, 4, S], BF16)
            va = pp.tile([128, 16, 8, DH + 1], BF16)  # [skp, sko, h, dh|1]
            on_ = pp.tile([128, 4, S], BF16)   # renormed out^T  [cp, co, s]
            negI = pp.tile([128, 128], BF16)
            ltB2 = pp.tile([128, 256], BF16)
            bqs = pp.tile([128, 4], F32)
            bks = pp.tile([128, 4], F32)
            bps = pp.tile([128, 8], F32)

            xbT_r = xbT_d.rearrange("(do dp) s -> dp do s", dp=128)
            wq_r = wqT_d.rearrange("(do dp) f -> dp do f", dp=128)
            wk_r = wkT_d.rearrange("(do dp) f -> dp do f", dp=128)
            wv_r = wvT_d.rearrange("(do dp) f -> dp do f", dp=128)
            wp_r = wpT_d.rearrange("(co cp) j -> cp co j", cp=128)
            xtiles = {}

            def emit_xdma(sb, split=False):
                xb = xp.tile([128, 8, 512], BF16)
                ssl = slice(sb * 512, (sb + 1) * 512)
                if split:  # head-phase blocks ride the HW rings, in order
                    nc.sync.dma_start(xb[:, 0:4, :], xbT_r[:, 0:4, ssl])
                    nc.scalar.dma_start(xb[:, 4:8, :], xbT_r[:, 4:8, ssl])
                else:
                    nc.gpsimd.dma_start(xb[:], xbT_r[:, :, ssl])
                xtiles[sb] = xb

            # only 3 DMA queues exist: gpsimd (software DGE; slow to start,
            # ~1us per descriptor batch), sync(SP) and scalar(Activation)
            # (hardware DGE, live at ~2.5us).  The 16 DMA engines drain all
            # rings round-robin at descriptor granularity, so aggregate
            # bandwidth (~440GB/s) is shared by whatever is in flight: the
            # lever for latency is ring ORDER.  Critical prefix xb0+wq
            # first on the two HW rings, then wk, wv, wp; the gpsimd ring
            # only prefetches xb1 (xb2/xb3 deferred by xin bufs=2
            # back-pressure).
            nc.scalar.dma_start(bqs[:], bqs_d[:])
            nc.scalar.dma_start(bks[:], bks_d[:])
            nc.scalar.dma_start(negI[:], negI_d[:])
            nc.scalar.dma_start(ltB2[:], ltB2_d[:])
            nc.scalar.dma_start(bps[:], bps_d[:])
            emit_xdma(0, split=True)
            nc.sync.dma_start(wq[:, 0:4, :], wq_r[:, 0:4, :])
            nc.scalar.dma_start(wq[:, 4:8, :], wq_r[:, 4:8, :])
            nc.sync.dma_start(wk[:, 0:4, :], wk_r[:, 0:4, :])
            nc.scalar.dma_start(wk[:, 4:8, :], wk_r[:, 4:8, :])
            nc.sync.dma_start(wv[:, 0:4, :], wv_r[:, 0:4, :])
            nc.scalar.dma_start(wv[:, 4:8, :], wv_r[:, 4:8, :])
            nc.sync.dma_start(wp[:, 0:2, :], wp_r[:, 0:2, :])
            nc.scalar.dma_start(wp[:, 2:4, :], wp_r[:, 2:4, :])
            nc.vector.memset(va[:, :, :, DH:DH + 1], 1.0)

            # ---- filler: split B/proj work into ~0.9us pieces ----
            done = set()
            fq = deque()
            bslot = {}

            def qk_piece(sb, which, ft, half):
                w_sb, dst, bias = ((wq, qt, bqs) if which == 'q'
                                   else (wk, kt, bks))
                key = (which, sb, ft)
                if half == 0:
                    bslot[key] = ps_pair.tile([128, 1024], F32, space="PSUM",
                                              tag="pair", name="pst")
                ps = bslot[key][:, 0:512]
                for do in (range(0, 4) if half == 0 else range(4, 8)):
                    nc.tensor.matmul(
                        ps, w_sb[:, do, ft * 128:(ft + 1) * 128],
                        xtiles[sb][:, do, :], start=(do == 0), stop=(do == 7))
                if half == 1:
                    nc.vector.tensor_scalar_add(
                        dst[:, ft, sb * 512:(sb + 1) * 512], ps,
                        bias[:, ft:ft + 1])
                    del bslot[key]

            def v_piece(sb, st, half):
                key = ('v', sb, st)
                if half == 0:
                    bslot[key] = ps_pair.tile([128, 1024], F32, space="PSUM",
                                              tag="pair", name="pst")
                ps = bslot[key][:, 0:512]
                for do in (range(0, 4) if half == 0 else range(4, 8)):
                    nc.tensor.matmul(
                        ps, xtiles[sb][:, do, st * 128:(st + 1) * 128],
                        wv[:, do, :], start=(do == 0), stop=(do == 7))
                if half == 1:
                    nc.vector.tensor_copy(
                        va[:, sb * 4 + st, :, :DH],
                        ps.rearrange("p (h d) -> p h d", h=8))
                    del bslot[key]

            def proj_piece(qb, jt):
                pst = ps_pair.tile([128, 1024], F32, space="PSUM", tag="pair")
                psj = pst[:, 0:512]
                for co in range(4):
                    nc.tensor.matmul(
                        psj, wp[:, co, jt * 128:(jt + 1) * 128],
                        on_[:, co, qb * 512:(qb + 1) * 512],
                        start=(co == 0), stop=(co == 3))
                ot_sb = op.tile([128, 512], F32, tag="o")
                nc.vector.tensor_scalar_add(ot_sb[:], psj, bps[:, jt:jt + 1])
                nc.sync.dma_start(
                    out_d[jt * 128:(jt + 1) * 128, qb * 512:(qb + 1) * 512],
                    ot_sb[:])

            def add_qk(sb, ft):
                for which in ('q', 'k'):
                    fq.append((None, lambda s=sb, w=which, f=ft:
                               qk_piece(s, w, f, 0)))
                    fq.append(((which, sb, ft), lambda s=sb, w=which, f=ft:
                               qk_piece(s, w, f, 1)))

            def add_v(sb):
                for st in range(4):
                    fq.append((None, lambda s=sb, t=st: v_piece(s, t, 0)))
                    fq.append((('v', sb, st) if st < 3 else ('v', sb),
                               lambda s=sb, t=st: v_piece(s, t, 1)))

            def pop_emit(n):
                for _ in range(n):
                    if not fq:
                        return
                    key, fn = fq.popleft()
                    fn()
                    if key is not None:
                        done.add(key)

            def drain_until(keys):
                while not all(k in done for k in keys):
                    assert fq, f"filler empty but need {keys}"
                    pop_emit(1)

            # ---- bf16 score matmul for head h, k-tile t, q-block qb ----
            def mm_score(out_ap, h, t, qb, c0, start, stop):
                g2, j = h % 2, h // 2
                p0 = 64 * g2
                lhsT = kt[p0:p0 + 64, j, 128 * t:128 * t + 128]
                rhs = qt[p0:p0 + 64, j, qb * 512 + c0:(qb + 1) * 512]
                nc.tensor.matmul(out_ap, lhsT, rhs, start=start, stop=stop)

            # combined causal-mask MM: adds -30*ltB to the two diagonal
            # 128-col strips of the score pair tile in one N=256 matmul.
            # Strip offsets within the [128,1024] pair tile are 128*m0 and
            # 512+128*(m0+1) -> constant spacing 640.
            def emit_mask_pair(pt, m0):
                base = pt[:, 128 * m0:128 * m0 + 128]
                out_ap = AP(base.tensor, base.offset,
                            [[1024, 128], [640, 2], [1, 128]])
                nc.tensor.matmul(out_ap, negI[:], ltB2[:],
                                 start=False, stop=True,
                                 skip_group_check=True)

            # ---- attention for (q-block qb, head h) ----
            def emit_av(ot, qb, h, et, u, nt):
                for half in range(2):
                    t = 2 * u + half
                    m = t - 4 * qb
                    c0 = 0 if m < 0 else 128 * m
                    hsl = 512 * half
                    nc.tensor.matmul(
                        ot[0:DH + 1, c0:512], va[:, t, h, :],
                        et[:, hsl + c0:hsl + 512],
                        start=(t == 0), stop=(t == nt - 1),
                        skip_group_check=True)

            def gen_c(qb, h):
                nt = 4 * qb + 4
                qsl = slice(qb * 512, (qb + 1) * 512)
                ot = None  # allocated lazily at first attn@V (avoids a
                # boundary stall on the ot-pool WAR with the previous
                # head-pair's pending renorm)
                pend = deque()  # (et, u) with attn@V deferred two rounds
                for u in range(nt // 2):
                    pt = ps_pair.tile([128, 1024], F32, space="PSUM", tag="pair")
                    et = ep.tile([128, 1024], BF16, tag="e")
                    m0 = 2 * u - 4 * qb
                    for half in range(2):
                        t = 2 * u + half
                        m = t - 4 * qb
                        hsl = 512 * half
                        if m < 0:  # full tile
                            mm_score(pt[:, hsl:hsl + 512], h, t, qb, 0,
                                     True, True)
                        else:      # diagonal tile: trim cols, mask strip later
                            c0 = 128 * m
                            mm_score(pt[:, hsl + c0:hsl + 512], h, t, qb, c0,
                                     True, False)
                    if m0 >= 0:
                        emit_mask_pair(pt, m0)
                    yield
                    # exp (ACT), batched over the pair when both halves full
                    if m0 < 0:
                        nc.scalar.activation(et[:], pt[:], AF.Exp)
                    else:
                        c0a, c0b = 128 * m0, 128 * (m0 + 1)
                        nc.scalar.activation(
                            et[:, c0a:512], pt[:, c0a:512], AF.Exp)
                        nc.scalar.activation(
                            et[:, 512 + c0b:1024], pt[:, 512 + c0b:1024],
                            AF.Exp)
                    pend.append((et, u))
                    if len(pend) > 2:
                        if ot is None:
                            ot = ps_ot.tile([DH + 1, 512], F32, space="PSUM",
                                            tag="ot")
                        e0, u0 = pend.popleft()
                        emit_av(ot, qb, h, e0, u0, nt)
                    yield
                while pend:
                    if ot is None:
                        ot = ps_ot.tile([DH + 1, 512], F32, space="PSUM",
                                        tag="ot")
                    e0, u0 = pend.popleft()
                    emit_av(ot, qb, h, e0, u0, nt)
                # softmax renorm: divide by ones-column row of ot
                dn = sp.tile([1, 512], F32, tag="dn")
                nc.vector.tensor_copy(dn[:], ot[DH:DH + 1, :])
                rec = sp.tile([1, 512], F32, tag="rec")
                nc.vector.reciprocal_approx_fast(rec[:], dn[:])
                rb = sp.tile([DH, 512], F32, tag="rb")
                nc.gpsimd.partition_broadcast(rb[:], rec[:])
                r0 = 64 * (h % 2)
                dst = on_[r0:r0 + 64, h // 2, qsl]
                nc.vector.tensor_tensor(dst, ot[0:DH, :], rb[:], ALU.mult)

            slot = {'i': 0, 'skip': 0, 'every': 1, 'pop': 1}

            def emit_c_pair(qb, h0, h1):
                gens = [gen_c(qb, h0), gen_c(qb, h1)]
                alive = [True, True]
                step = 0
                while any(alive):
                    for i in (0, 1):
                        if alive[i]:
                            try:
                                next(gens[i])
                            except StopIteration:
                                alive[i] = False
                    step += 1
                    if step % 2 == 0:
                        slot['i'] += 1
                        if (slot['i'] > slot['skip']
                                and (slot['i'] - slot['skip']) % slot['every'] == 0):
                            pop_emit(slot['pop'])

            # ---- emission schedule ----
            # critical prefix of B(0): all four Q chains first (bridges PE
            # over the wk/wv DMA arrival), then K ft0 and V
            for ft in range(4):
                qk_piece(0, 'q', ft, 0); qk_piece(0, 'q', ft, 1)
                done.add(('q', 0, ft))
            qk_piece(0, 'k', 0, 0); qk_piece(0, 'k', 0, 1); done.add(('k', 0, 0))
            for st in range(4):
                v_piece(0, st, 0); v_piece(0, st, 1)
            done.add(('v', 0))
            for ft in range(1, 4):
                fq.append((None, lambda f=ft: qk_piece(0, 'k', f, 0)))
                fq.append((('k', 0, ft), lambda f=ft: qk_piece(0, 'k', f, 1)))

            # injection config per qb: skip slots after the qb boundary so
            # injected pieces never stall the in-order PE queue on fresh
            # renorm/DMA deps
            inject = {0: (1, 1, 3), 1: (2, 1, 2), 2: (2, 1, 2), 3: (2, 4, 1)}
            for qb in range(4):
                slot['i'] = 0
                slot['skip'], slot['every'], slot['pop'] = inject[qb]
                if qb >= 1:  # proj pieces first: safe once prev qb renormed
                    for jt in range(8):
                        fq.append((None, lambda q=qb - 1, j=jt:
                                   proj_piece(q, j)))
                if qb < 3:
                    emit_xdma(qb + 1, split=(qb == 0))
                    for ft in range(4):
                        add_qk(qb + 1, ft)
                        if ft == 0:
                            add_v(qb + 1)
                for hp in range(4):
                    need = [('q', qb, hp), ('k', qb, hp)]
                    if hp == 0:
                        need.append(('v', qb))
                    drain_until(need)
                    emit_c_pair(qb, 2 * hp, 2 * hp + 1)
            pop_emit(len(fq))
            for jt in range(8):
                proj_piece(3, jt)

    nc.compile()
    return nc


def kernel(x, Wq, bq, Wk, bk, Wv, bv, Wp, bp):
    global _NC
    if _NC is None:
        _NC = _build()

    x = np.asarray(x, np.float32)
    Wq, bq = np.asarray(Wq, np.float32), np.asarray(bq, np.float32)
    Wk, bk = np.asarray(Wk, np.float32), np.asarray(bk, np.float32)
    Wv, bv = np.asarray(Wv, np.float32), np.asarray(bv, np.float32)
    Wp, bp = np.asarray(Wp, np.float32), np.asarray(bp, np.float32)

    bf = ml_dtypes.bfloat16
    negI = np.ascontiguousarray((-30.0 * np.eye(128, dtype=np.float32)).astype(bf))
    i_ = np.arange(128)
    ltB = (i_[None, :] < i_[:, None]).astype(np.float32).astype(bf)
    ltB2 = np.ascontiguousarray(np.concatenate([ltB, ltB], axis=1))

    xbT = [np.ascontiguousarray(x[b].T.astype(bf)) for b in range(B)]

    in_maps = []
    for c in range(N_CORES):
        b, g = c // 2, c % 2
        hs = range(8 * g, 8 * g + 8)
        kidx = np.array([dh * 16 + h for h in hs for dh in range(DH)])
        fsl = slice(FG * g, FG * (g + 1))
        bp_c = (bp if g == 0 else 0.0) + Wp[:, fsl] @ bv[fsl]
        in_maps.append({
            "xbT": xbT[b],
            "wqT": np.ascontiguousarray((SCALE * Wq[fsl].T).astype(bf)),
            "wkT": np.ascontiguousarray(Wk[kidx].T.astype(bf)),
            "wvT": np.ascontiguousarray(Wv[fsl].T.astype(bf)),
            "wpT": np.ascontiguousarray(Wp[:, fsl].T.astype(bf)),
            "bqs": np.ascontiguousarray((SCALE * bq[fsl]).reshape(4, 128).T),
            "bks": np.ascontiguousarray(bk[kidx].reshape(4, 128).T),
            "bps": np.ascontiguousarray(bp_c.reshape(8, 128).T.astype(np.float32)),
            "negI": negI,
            "ltB2": ltB2,
        })

    res = run_bass_kernel_spmd(_NC, in_maps, core_ids=list(range(N_CORES)))
    out = np.empty((B, S, D), np.float32)
    for b in range(B):
        acc = res.results[2 * b]["outT"] + res.results[2 * b + 1]["outT"]
        out[b] = acc.T
    return out


# revision 26
# speedup vs baseline: 1.1395x; 1.0780x over previous
"""Multi-head self-attention (B=4,S=2048,D=1024,H=16,DH=64, causal) on 8 trn2 cores.

Sharding: core c -> batch b=c//2, head-group g=c%2 (8 heads each).

v2 restructure over the 312us baseline (PE busy was 268us with cold-clock
inflation, 19.4us DMA head, serial qb-boundary stalls):
- DMA head: x block + weights split across 5 engine DMA queues in parallel
  (was serial on one queue; first matmul at 19.4us -> target ~9us).
- Causal-mask matmuls pair-combined: one [128x256] MM per diagonal PSUM
  pair via a strided 3D out AP (was 2x [128x128] MMs).
- Emission engine: QKV-projection and output-projection work is split into
  ~0.9us pieces held in a filler deque and injected between attention
  u-rounds, keeping PE dense while ACT (exp) catches up; PRE-drains before
  each head-pair guarantee dependencies.
- Scores in bf16 (fp8 fails numerics: e4m3 anywhere pushes rel err to
  3e-2..8e-2 vs the 2e-2 gate; measured in numpy sim).
- exp on ACT batched over k-tile pairs ([128,1024] two-bank PSUM reads).
- attn@V in bf16 with ones-column denominator; renorm via
  reciprocal_approx_fast (DVE) + partition_broadcast (gpsimd) + mult (DVE).
- bv and bp folded host-side into the output-projection bias.

K-projection quirk (reference views k as (B,S,DH,H)): head h uses Wk rows
[dh*16+h for dh in range(64)] -- handled by host-side row gather.
"""
import numpy as np
import ml_dtypes
from collections import deque

import concourse.mybir as mybir
import concourse.tile as tile
from concourse import bacc
from concourse.ap import AP
from concourse.bass_utils import run_bass_kernel_spmd

F32 = mybir.dt.float32
BF16 = mybir.dt.bfloat16
AF = mybir.ActivationFunctionType
ALU = mybir.AluOpType

B, S, D, H, DH = 4, 2048, 1024, 16, 64
FG = 512          # features per head-group (8 heads * 64)
N_CORES = 8
SCALE = 0.125     # 1/sqrt(64)

_NC = None


def _build():
    nc = bacc.Bacc("TRN2", target_bir_lowering=False, debug=False,
                   num_devices=N_CORES, enable_asserts=False)
    xbT_d = nc.dram_tensor("xbT", [D, S], BF16, kind="ExternalInput").ap()
    wqT_d = nc.dram_tensor("wqT", [D, FG], BF16, kind="ExternalInput").ap()
    wkT_d = nc.dram_tensor("wkT", [D, FG], BF16, kind="ExternalInput").ap()
    wvT_d = nc.dram_tensor("wvT", [D, FG], BF16, kind="ExternalInput").ap()
    wpT_d = nc.dram_tensor("wpT", [FG, D], BF16, kind="ExternalInput").ap()
    bqs_d = nc.dram_tensor("bqs", [128, 4], F32, kind="ExternalInput").ap()
    bks_d = nc.dram_tensor("bks", [128, 4], F32, kind="ExternalInput").ap()
    bps_d = nc.dram_tensor("bps", [128, 8], F32, kind="ExternalInput").ap()
    ltB2_d = nc.dram_tensor("ltB2", [128, 256], BF16, kind="ExternalInput").ap()
    out_d = nc.dram_tensor("outT", [D, S], F32, kind="ExternalOutput").ap()

    with tile.TileContext(nc) as tc:
        with tc.tile_pool(name="persist", bufs=1) as pp, \
             tc.tile_pool(name="xin", bufs=2) as xp, \
             tc.tile_pool(name="etile", bufs=8) as ep, \
             tc.tile_pool(name="small", bufs=4) as sp, \
             tc.tile_pool(name="outtile", bufs=4) as op, \
             tc.tile_pool(name="pspair", bufs=3, space="PSUM") as ps_pair, \
             tc.tile_pool(name="psot", bufs=2, space="PSUM") as ps_ot:

            # ---- persistent SBUF tensors ----
            wq = pp.tile([128, 8, FG], BF16)   # [dp, do, f]  (pre-scaled 1/8)
            wk = pp.tile([128, 8, FG], BF16)
            wv = pp.tile([128, 8, FG], BF16)
            wp = pp.tile([128, 4, D], BF16)    # [cp, co, j]
            qt = pp.tile([128, 4, S], BF16)    # [fp, fo, s]
            kt = pp.tile([128, 4, S], BF16)
            va = pp.tile([128, 16, 8, DH + 1], BF16)  # [skp, sko, h, dh|1]
            on_ = pp.tile([128, 4, S], BF16)   # renormed out^T  [cp, co, s]
            ltB2 = pp.tile([128, 256], BF16)
            bqs = pp.tile([128, 4], F32)
            bks = pp.tile([128, 4], F32)
            bps = pp.tile([128, 8], F32)

            xbT_r = xbT_d.rearrange("(do dp) s -> dp do s", dp=128)
            wq_r = wqT_d.rearrange("(do dp) f -> dp do f", dp=128)
            wk_r = wkT_d.rearrange("(do dp) f -> dp do f", dp=128)
            wv_r = wvT_d.rearrange("(do dp) f -> dp do f", dp=128)
            wp_r = wpT_d.rearrange("(co cp) j -> cp co j", cp=128)
            xtiles = {}

            def emit_xdma(sb, split=False):
                xb = xp.tile([128, 8, 512], BF16)
                ssl = slice(sb * 512, (sb + 1) * 512)
                if split:  # head-phase blocks ride the HW rings, in order
                    nc.sync.dma_start(xb[:, 0:4, :], xbT_r[:, 0:4, ssl])
                    nc.scalar.dma_start(xb[:, 4:8, :], xbT_r[:, 4:8, ssl])
                else:
                    nc.gpsimd.dma_start(xb[:], xbT_r[:, :, ssl])
                xtiles[sb] = xb

            # only 3 DMA queues exist: gpsimd (software DGE; slow to start,
            # ~1us per descriptor batch), sync(SP) and scalar(Activation)
            # (hardware DGE, live at ~2.5us).  The 16 DMA engines drain all
            # rings round-robin at descriptor granularity, so aggregate
            # bandwidth (~440GB/s) is shared by whatever is in flight: the
            # lever for latency is ring ORDER.  Critical prefix xb0+wq
            # first on the two HW rings, then wk, wv, wp; the gpsimd ring
            # only prefetches xb1 (xb2/xb3 deferred by xin bufs=2
            # back-pressure).
            nc.scalar.dma_start(bqs[:], bqs_d[:])
            nc.scalar.dma_start(bks[:], bks_d[:])
            nc.scalar.dma_start(ltB2[:], ltB2_d[:])
            nc.scalar.dma_start(bps[:], bps_d[:])
            emit_xdma(0, split=True)
            # wq/wk f-sliced so each Q/K chain's weight block arrives
            # just-in-time behind xb0
            nc.sync.dma_start(wq[:, :, 0:128], wq_r[:, :, 0:128])
            nc.scalar.dma_start(wq[:, :, 128:256], wq_r[:, :, 128:256])
            nc.sync.dma_start(wq[:, :, 256:384], wq_r[:, :, 256:384])
            nc.scalar.dma_start(wq[:, :, 384:512], wq_r[:, :, 384:512])
            nc.sync.dma_start(wk[:, :, 0:128], wk_r[:, :, 0:128])
            nc.scalar.dma_start(wv[:, 4:8, :], wv_r[:, 4:8, :])
            nc.sync.dma_start(wv[:, 0:4, :], wv_r[:, 0:4, :])
            nc.scalar.dma_start(wk[:, :, 128:256], wk_r[:, :, 128:256])
            nc.sync.dma_start(wk[:, :, 256:384], wk_r[:, :, 256:384])
            nc.scalar.dma_start(wk[:, :, 384:512], wk_r[:, :, 384:512])
            nc.sync.dma_start(wp[:, 0:2, :], wp_r[:, 0:2, :])
            nc.scalar.dma_start(wp[:, 2:4, :], wp_r[:, 2:4, :])
            nc.vector.memset(va[:, :, :, DH:DH + 1], 1.0)

            # ---- filler: split B/proj work into ~0.9us pieces ----
            done = set()
            fq = deque()
            bslot = {}

            def qk_piece(sb, which, ft, half):
                w_sb, dst, bias = ((wq, qt, bqs) if which == 'q'
                                   else (wk, kt, bks))
                key = (which, sb, ft)
                if half == 0:
                    bslot[key] = ps_pair.tile([128, 1024], F32, space="PSUM",
                                              tag="pair", name="pst")
                ps = bslot[key][:, 0:512]
                for do in (range(0, 4) if half == 0 else range(4, 8)):
                    nc.tensor.matmul(
                        ps, w_sb[:, do, ft * 128:(ft + 1) * 128],
                        xtiles[sb][:, do, :], start=(do == 0), stop=(do == 7))
                if half == 1:
                    nc.vector.tensor_scalar_add(
                        dst[:, ft, sb * 512:(sb + 1) * 512], ps,
                        bias[:, ft:ft + 1])
                    del bslot[key]

            def v_piece(sb, st, half):
                key = ('v', sb, st)
                if half == 0:
                    bslot[key] = ps_pair.tile([128, 1024], F32, space="PSUM",
                                              tag="pair", name="pst")
                ps = bslot[key][:, 0:512]
                for do in (range(0, 4) if half == 0 else range(4, 8)):
                    nc.tensor.matmul(
                        ps, xtiles[sb][:, do, st * 128:(st + 1) * 128],
                        wv[:, do, :], start=(do == 0), stop=(do == 7))
                if half == 1:
                    nc.vector.tensor_copy(
                        va[:, sb * 4 + st, :, :DH],
                        ps.rearrange("p (h d) -> p h d", h=8))
                    del bslot[key]

            def proj_piece(qb, jt):
                pst = ps_pair.tile([128, 1024], F32, space="PSUM", tag="pair")
                psj = pst[:, 0:512]
                for co in range(4):
                    nc.tensor.matmul(
                        psj, wp[:, co, jt * 128:(jt + 1) * 128],
                        on_[:, co, qb * 512:(qb + 1) * 512],
                        start=(co == 0), stop=(co == 3))
                ot_sb = op.tile([128, 512], F32, tag="o")
                nc.vector.tensor_scalar_add(ot_sb[:], psj, bps[:, jt:jt + 1])
                nc.sync.dma_start(
                    out_d[jt * 128:(jt + 1) * 128, qb * 512:(qb + 1) * 512],
                    ot_sb[:])

            def add_qk(sb, ft):
                for which in ('q', 'k'):
                    fq.append((None, lambda s=sb, w=which, f=ft:
                               qk_piece(s, w, f, 0)))
                    fq.append(((which, sb, ft), lambda s=sb, w=which, f=ft:
                               qk_piece(s, w, f, 1)))

            def add_v(sb):
                for st in range(4):
                    fq.append((None, lambda s=sb, t=st: v_piece(s, t, 0)))
                    fq.append((('v', sb, st) if st < 3 else ('v', sb),
                               lambda s=sb, t=st: v_piece(s, t, 1)))

            def pop_emit(n):
                for _ in range(n):
                    if not fq:
                        return
                    key, fn = fq.popleft()
                    fn()
                    if key is not None:
                        done.add(key)

            def drain_until(keys):
                while not all(k in done for k in keys):
                    assert fq, f"filler empty but need {keys}"
                    pop_emit(1)

            # ---- bf16 score matmul for head h, k-tile t, q-block qb ----
            def mm_score(out_ap, h, t, qb, c0, start, stop):
                g2, j = h % 2, h // 2
                p0 = 64 * g2
                lhsT = kt[p0:p0 + 64, j, 128 * t:128 * t + 128]
                rhs = qt[p0:p0 + 64, j, qb * 512 + c0:(qb + 1) * 512]
                nc.tensor.matmul(out_ap, lhsT, rhs, start=start, stop=stop)

            # causal mask on DVE: zero the above-diagonal part of the two
            # 128-col diagonal strips of an exp'd pair tile with one
            # strided tensor_tensor multiply (strips sit 640 cols apart in
            # the [128,1024] et tile; ltB2 holds the strip mask twice).
            def emit_mask_pair(et, m0):
                base = et[:, 128 * m0:128 * m0 + 128]
                strip = AP(base.tensor, base.offset,
                           [[1024, 128], [640, 2], [1, 128]])
                nc.vector.tensor_tensor(
                    strip, strip,
                    ltB2[:].rearrange("p (a b) -> p a b", a=2), ALU.mult)

            # ---- attention for (q-block qb, head h) ----
            def emit_av(ot, qb, h, et, u, nt):
                for half in range(2):
                    t = 2 * u + half
                    m = t - 4 * qb
                    c0 = 0 if m < 0 else 128 * m
                    hsl = 512 * half
                    nc.tensor.matmul(
                        ot[0:DH + 1, c0:512], va[:, t, h, :],
                        et[:, hsl + c0:hsl + 512],
                        start=(t == 0), stop=(t == nt - 1),
                        skip_group_check=True)

            def gen_c(qb, h):
                nt = 4 * qb + 4
                qsl = slice(qb * 512, (qb + 1) * 512)
                ot = None  # allocated lazily at first attn@V (avoids a
                # boundary stall on the ot-pool WAR with the previous
                # head-pair's pending renorm)
                prev = None  # (et, u) whose attn@V is deferred one round
                for u in range(nt // 2):
                    pt = ps_pair.tile([128, 1024], F32, space="PSUM", tag="pair")
                    et = ep.tile([128, 1024], BF16, tag="e")
                    m0 = 2 * u - 4 * qb
                    for half in range(2):
                        t = 2 * u + half
                        m = t - 4 * qb
                        hsl = 512 * half
                        c0 = 0 if m < 0 else 128 * m
                        mm_score(pt[:, hsl + c0:hsl + 512], h, t, qb, c0,
                                 True, True)
                    yield
                    # exp (ACT), batched over the pair when both halves full
                    if m0 < 0:
                        nc.scalar.activation(et[:], pt[:], AF.Exp)
                    else:
                        c0a, c0b = 128 * m0, 128 * (m0 + 1)
                        nc.scalar.activation(
                            et[:, c0a:512], pt[:, c0a:512], AF.Exp)
                        nc.scalar.activation(
                            et[:, 512 + c0b:1024], pt[:, 512 + c0b:1024],
                            AF.Exp)
                        emit_mask_pair(et, m0)
                    if prev is not None:
                        if ot is None:
                            ot = ps_ot.tile([DH + 1, 512], F32, space="PSUM",
                                            tag="ot")
                        emit_av(ot, qb, h, prev[0], prev[1], nt)
                    prev = (et, u)
                    yield
                if ot is None:
                    ot = ps_ot.tile([DH + 1, 512], F32, space="PSUM",
                                    tag="ot")
                emit_av(ot, qb, h, prev[0], prev[1], nt)
                # softmax renorm: divide by ones-column row of ot
                dn = sp.tile([1, 512], F32, tag="dn")
                nc.vector.tensor_copy(dn[:], ot[DH:DH + 1, :])
                rec = sp.tile([1, 512], F32, tag="rec")
                nc.vector.reciprocal_approx_fast(rec[:], dn[:])
                rb = sp.tile([DH, 512], F32, tag="rb")
                nc.gpsimd.partition_broadcast(rb[:], rec[:])
                r0 = 64 * (h % 2)
                dst = on_[r0:r0 + 64, h // 2, qsl]
                nc.vector.tensor_tensor(dst, ot[0:DH, :], rb[:], ALU.mult)

            slot = {'i': 0, 'skip': 0, 'every': 1, 'pop': 1}

            def emit_c_pair(qb, h0, h1):
                gens = [gen_c(qb, h0), gen_c(qb, h1)]
                alive = [True, True]
                step = 0
                while any(alive):
                    for i in (0, 1):
                        if alive[i]:
                            try:
                                next(gens[i])
                            except StopIteration:
                                alive[i] = False
                    step += 1
                    if step % 2 == 0:
                        slot['i'] += 1
                        if (slot['i'] > slot['skip']
                                and (slot['i'] - slot['skip']) % slot['every'] == 0):
                            pop_emit(slot['pop'])

            # ---- emission schedule ----
            # critical prefix of B(0): all four Q chains first (bridges PE
            # over the wk/wv DMA arrival), then K ft0 and V
            for ft in range(4):
                qk_piece(0, 'q', ft, 0); qk_piece(0, 'q', ft, 1)
                done.add(('q', 0, ft))
            qk_piece(0, 'k', 0, 0); qk_piece(0, 'k', 0, 1); done.add(('k', 0, 0))
            for st in range(4):
                v_piece(0, st, 0); v_piece(0, st, 1)
            done.add(('v', 0))
            for ft in range(1, 4):
                fq.append((None, lambda f=ft: qk_piece(0, 'k', f, 0)))
                fq.append((('k', 0, ft), lambda f=ft: qk_piece(0, 'k', f, 1)))

            # injection config per qb: skip slots after the qb boundary so
            # injected pieces never stall the in-order PE queue on fresh
            # renorm/DMA deps
            inject = {0: (1, 1, 3), 1: (2, 1, 2), 2: (2, 1, 2), 3: (2, 4, 1)}
            for qb in range(4):
                slot['i'] = 0
                slot['skip'], slot['every'], slot['pop'] = inject[qb]
                if qb >= 1:  # proj pieces first: safe once prev qb renormed
                    for jt in range(8):
                        fq.append((None, lambda q=qb - 1, j=jt:
                                   proj_piece(q, j)))
                if qb < 3:
                    emit_xdma(qb + 1, split=(qb == 0))
                    for ft in range(4):
                        add_qk(qb + 1, ft)
                        if ft == 0:
                            add_v(qb + 1)
                for hp in range(4):
                    need = [('q', qb, hp), ('k', qb, hp)]
                    if hp == 0:
                        need.append(('v', qb))
                    drain_until(need)
                    emit_c_pair(qb, 2 * hp, 2 * hp + 1)
            pop_emit(len(fq))
            for jt in range(8):
                proj_piece(3, jt)

    nc.compile()
    return nc


def kernel(x, Wq, bq, Wk, bk, Wv, bv, Wp, bp):
    global _NC
    if _NC is None:
        _NC = _build()

    x = np.asarray(x, np.float32)
    Wq, bq = np.asarray(Wq, np.float32), np.asarray(bq, np.float32)
    Wk, bk = np.asarray(Wk, np.float32), np.asarray(bk, np.float32)
    Wv, bv = np.asarray(Wv, np.float32), np.asarray(bv, np.float32)
    Wp, bp = np.asarray(Wp, np.float32), np.asarray(bp, np.float32)

    bf = ml_dtypes.bfloat16
    i_ = np.arange(128)
    # keep-mask for the DVE multiply: strip element [k_row, q_col] survives
    # iff q >= k within the diagonal 128x128 strip
    keep = (i_[None, :] >= i_[:, None]).astype(np.float32).astype(bf)
    ltB2 = np.ascontiguousarray(np.concatenate([keep, keep], axis=1))

    xbT = [np.ascontiguousarray(x[b].T.astype(bf)) for b in range(B)]

    in_maps = []
    for c in range(N_CORES):
        b, g = c // 2, c % 2
        hs = range(8 * g, 8 * g + 8)
        kidx = np.array([dh * 16 + h for h in hs for dh in range(DH)])
        fsl = slice(FG * g, FG * (g + 1))
        bp_c = (bp if g == 0 else 0.0) + Wp[:, fsl] @ bv[fsl]
        in_maps.append({
            "xbT": xbT[b],
            "wqT": np.ascontiguousarray((SCALE * Wq[fsl].T).astype(bf)),
            "wkT": np.ascontiguousarray(Wk[kidx].T.astype(bf)),
            "wvT": np.ascontiguousarray(Wv[fsl].T.astype(bf)),
            "wpT": np.ascontiguousarray(Wp[:, fsl].T.astype(bf)),
            "bqs": np.ascontiguousarray((SCALE * bq[fsl]).reshape(4, 128).T),
            "bks": np.ascontiguousarray(bk[kidx].reshape(4, 128).T),
            "bps": np.ascontiguousarray(bp_c.reshape(8, 128).T.astype(np.float32)),
            "ltB2": ltB2,
        })

    res = run_bass_kernel_spmd(_NC, in_maps, core_ids=list(range(N_CORES)))
    out = np.empty((B, S, D), np.float32)
    for b in range(B):
        acc = res.results[2 * b]["outT"] + res.results[2 * b + 1]["outT"]
        out[b] = acc.T
    return out
